# revision 22
# baseline (speedup 1.0000x reference)
"""Top-1 MoE (BmmMoeModel) on 8 Trainium2 NeuronCores.

Strategy: expert-parallel with routing-aware dispatch.
  - Host computes the router (x @ gate_w.T, argmax, sigmoid) -- 0.26% of the
    model FLOPs -- and uses it as the sharding function: each core receives
    only the tokens routed to its expert (scaled by the routing weight,
    transposed to [H, C], cast to bf16) plus that expert's weights.
  - The final "sum over experts" is a disjoint scatter of each core's token
    outputs back into the full [T, H] output on the host (top-1 routing means
    non-selected experts contribute exactly zero).

On-device layout: both GEMMs run with the *activations as the stationary
operand* and the weights as the moving operand.  A stationary tile is reused
for 4 consecutive matmuls (the 4 512-wide PSUM banks covering the weight's
output columns), so the PE's between-matmul weight-reload bubble (~46 ns per
matmul when the stationary changes every matmul, measured) is amortized away
and the PE runs at its row-streaming rate.

  Phase A  gu[c,f] = sum_k x[c,k] wgu[k,f]: stationary = xsT[k-chunk,c-chunk]
           ([128,128]), moving = wgu[k-chunk,:] in 4 q-slices of 512.  PSUM
           out is [tokens, FF2] = 4 banks/c-chunk -> two passes of 2 c-chunks.
  silu-glu from PSUM (fp32): act[c, i] = up * silu(gate), stored bf16.
  transpose act [c,i] -> actT [i,c] with the DMA xbar (dma_start_transpose),
           off the critical engines.
  Phase B  out[c,h] = sum_i act[c,i] wdn[i,h]: stationary = actT[i-chunk,
           c-chunk], moving = wdn[i-chunk,:] in 4 q-slices.  Output lands
           token-major [C, H] fp32 -> contiguous 8 KB DMA rows and a
           transpose-free host scatter.

Weights stream k-tile-by-k-tile on the sync queue in the order Phase A
consumes them (accumulation over k is arrival-ordered); activations ride the
scalar-engine queue, outputs the gpsimd queue, so no DMA issue stream gates
another.  Matmuls run bf16 with fp32 PSUM accumulation (fp8 was measured at
5-6.5% relative error on this problem -- over the 2e-2 gate -- and rejected).
"""

import numpy as np
import ml_dtypes

B, S, H, I, E = 2, 2048, 2048, 1024, 8
T = B * S
FF2 = 2 * I
C = 512          # per-expert device token capacity (4 c-chunks of 128)

# Stash of the last run's BassKernelResults (for test harness introspection).
LAST = {}
_PROGRAM_CACHE = {}


def _build_program():
    import concourse.bass as bass
    import concourse.mybir as mybir
    import concourse.tile as tile
    from concourse import bacc

    dt = mybir.dt
    AF = mybir.ActivationFunctionType

    nc = bacc.Bacc(None, target_bir_lowering=False)
    # xsP[p, k*C + c] = x_tokens[c, k*128 + p]: partition-major so each DMA
    # row run is 4 KB (k-group of 4) instead of the 1 KB a plain [H, C]
    # layout would give -- 1 KB packets are per-packet-overhead-bound and
    # were measured to stall Phase A.
    xsP = nc.dram_tensor("xsP", [128, (H // 128) * C], dt.bfloat16,
                         kind="ExternalInput")
    wgu = nc.dram_tensor("wgu", [H, FF2], dt.bfloat16, kind="ExternalInput")
    wdn = nc.dram_tensor("wdn", [I, H], dt.bfloat16, kind="ExternalInput")
    # output ships bf16 (host upcasts): halves the PSUM->SBUF copy time and
    # the output DMA bytes; adds ~0.2% RMS rounding vs the 2e-2 gate
    outC = nc.dram_tensor("outC", [C, H], dt.bfloat16, kind="ExternalOutput")

    KH = H // 128    # 16 contraction chunks for gate_up
    KI = I // 128    # 8 contraction chunks for down

    with tile.TileContext(nc) as tc:
        with (
            tc.tile_pool(name="res", bufs=1) as res,
            tc.tile_pool(name="work", bufs=1) as work,
            tc.tile_pool(name="psum", bufs=1, space=bass.MemorySpace.PSUM) as psum,
        ):
            # PE clock pre-warm: the HAM clock gate holds the PE at 1.2 GHz
            # until it has seen ~3.4 us of sustained activity; spin small
            # dependency-free matmuls so the ramp happens before real work.
            warm = work.tile([128, 128], dt.bfloat16, tag="warm", bufs=1,
                             name="warm")
            nc.gpsimd.memset(warm[:], 0.0)
            warm_ps = psum.tile([128, 64], dt.float32, tag="ps", bufs=8,
                                name="warm_ps")
            for _ in range(64):
                nc.tensor.matmul(warm_ps[0:32, :], warm[:, 0:32], warm[:, 0:64],
                                 start=True, stop=True)

            # All inputs ride ONE queue (sync), interleaved in exact
            # consumption order.  The DMA queues share the core's HBM
            # bandwidth, so a second concurrent input stream halves the
            # weight arrival rate and stalls Phase A (measured: +10 us);
            # strict FIFO on one queue keeps arrival == need order at full
            # bandwidth.  xs4[b] ([128,4,C], slice [:,j,:] = k-tile 4b+j)
            # is needed just before wg[4b].
            # The PE start is gated by the first stationary+moving tiles, so
            # the k=0 pieces are split small: xsA is one k-tile of tokens
            # (128 KB) and wg[0] arrives as two half-tiles.
            xsA = res.tile([128, 1, C], dt.bfloat16, tag="xsA", name="xsA")
            xsB = res.tile([128, 3, C], dt.bfloat16, tag="xsB", name="xsB")
            xs4 = [None] + [res.tile([128, 4, C], dt.bfloat16, tag=f"xs{b}",
                                     name=f"xs{b}") for b in range(1, 4)]
            wg0h = [res.tile([128, FF2 // 2], dt.bfloat16, tag=f"wg0{h}",
                             name=f"wg0{h}") for h in range(2)]
            wg = [None] + [res.tile([128, FF2], dt.bfloat16, tag=f"wg{k}",
                                    name=f"wg{k}") for k in range(1, KH)]
            xsP_r = xsP.rearrange("p (b j c) -> b p j c", b=4, j=4)

            def wg_dma(k):
                nc.sync.dma_start(wg[k][:], wgu[k * 128:(k + 1) * 128, :])

            nc.sync.dma_start(xsA[:], xsP_r[0][:, 0:1, :])
            nc.sync.dma_start(wg0h[0][:], wgu[0:128, 0:FF2 // 2])
            nc.sync.dma_start(wg0h[1][:], wgu[0:128, FF2 // 2:FF2])
            nc.sync.dma_start(xsB[:], xsP_r[0][:, 1:4, :])
            for k in range(1, 3):
                wg_dma(k)
            nc.sync.dma_start(xs4[1][:], xsP_r[1])
            for k in range(3, 8):
                wg_dma(k)
            nc.sync.dma_start(xs4[2][:], xsP_r[2])
            nc.sync.dma_start(xs4[3][:], xsP_r[3])
            for k in range(8, KH):
                wg_dma(k)

            def xs_sl(k, c):
                if k == 0:
                    return xsA[:, 0, c * 128:(c + 1) * 128]
                if k < 4:
                    return xsB[:, k - 1, c * 128:(c + 1) * 128]
                return xs4[k // 4][:, k % 4, c * 128:(c + 1) * 128]

            def wg_sl(k, q):
                if k == 0:
                    return wg0h[q // 2][:, (q % 2) * 512:(q % 2 + 1) * 512]
                return wg[k][:, q * 512:(q + 1) * 512]
            # down weight (Phase B moving): 2 tiles of 4 i-chunks each,
            # behind the Phase A stream.
            wd4 = []
            wdn_r = wdn.rearrange("(b j p) h -> b p j h", j=4, p=128)
            for b in range(2):
                t_ = res.tile([128, 4, H], dt.bfloat16, tag=f"wd{b}",
                              name=f"wd{b}")
                nc.sync.dma_start(t_[:], wdn_r[b])
                wd4.append(t_)

            act = [res.tile([128, I], dt.bfloat16, tag=f"act{c}",
                            name=f"act{c}") for c in range(4)]
            # actT[c][:, j, :] = act[c][:, j*128:(j+1)*128].T  (i-major)
            actT = [res.tile([128, KI, 128], dt.bfloat16, tag=f"actT{c}",
                             name=f"actT{c}") for c in range(4)]

            # Phase A, k-blocked with partial-sum spill: four blocks of
            # (c-pair x 8 k-steps).  Blocks over k=0..7 run while the second
            # half of the weight stream is still arriving and spill their
            # fp32 partials to SBUF; blocks over k=8..15 add the spill back
            # into PSUM (in place, on vector) before the silu-glu.  This
            # keeps the PE compute-bound through the weight-arrival window
            # instead of serializing a full-k pass behind the last weight
            # byte (~4 us).  Within each block the first c-chunk runs its
            # last SG k-steps alone so its spill/merge (the bank-freeing
            # consumers) overlap the second c-chunk's k-tail.
            sp = {(c, q): res.tile([128, 512], dt.float32, tag=f"sp{c}_{q}",
                                   name=f"sp{c}_{q}")
                  for c in range(4) for q in range(4)}
            KB = KH // 2
            SG = 3
            for half in range(2):
                ka, kb = half * KB, half * KB + KB
                for p in range(2):
                    cs = (2 * p, 2 * p + 1)
                    pa = {(c, q): psum.tile([128, 512], dt.float32, tag="ps",
                                            bufs=8, name=f"pa{half}{c}_{q}")
                          for c in cs for q in range(4)}

                    def a_mm(k, c):
                        lhs = xs_sl(k, c)
                        for q in range(4):
                            nc.tensor.matmul(
                                pa[(c, q)][:], lhs, wg_sl(k, q),
                                start=(k == ka), stop=(k == kb - 1),
                            )

                    def consume(c):
                        if half == 0:
                            # spill k0..7 partials, split scalar/vector
                            for q in range(4):
                                if q < 2:
                                    nc.scalar.activation(sp[(c, q)][:],
                                                         pa[(c, q)][:], AF.Copy)
                                else:
                                    nc.vector.tensor_copy(sp[(c, q)][:],
                                                          pa[(c, q)][:])
                        else:
                            # merge spill + silu-glu: up = q0,q1; gate = q2,q3
                            for h in range(2):
                                nc.vector.tensor_add(pa[(c, 2 + h)][:],
                                                     pa[(c, 2 + h)][:],
                                                     sp[(c, 2 + h)][:])
                                st = work.tile([128, 512], dt.float32,
                                               tag="silu", bufs=4,
                                               name=f"st{c}_{h}")
                                nc.scalar.activation(st[:], pa[(c, 2 + h)][:],
                                                     AF.Silu)
                                nc.vector.tensor_add(pa[(c, h)][:],
                                                     pa[(c, h)][:],
                                                     sp[(c, h)][:])
                                nc.vector.tensor_mul(
                                    act[c][:, h * 512:(h + 1) * 512],
                                    pa[(c, h)][:], st[:])
                            nc.sync.dma_start_transpose(actT[c][:], act[c][:])

                    for k in range(ka, kb - SG):
                        for c in cs:
                            a_mm(k, c)
                    for k in range(kb - SG, kb):
                        a_mm(k, cs[0])
                    consume(cs[0])
                    for k in range(kb - SG, kb):
                        a_mm(k, cs[1])
                    consume(cs[1])

            # Phase B: token-major out, one c-chunk at a time (4 banks).
            # The last c-chunk runs its output columns (q) outermost so the
            # trailing copy+DMA after the final matmul is only 512 wide.
            for c in range(4):
                if c < 3:
                    pb = [psum.tile([128, 512], dt.float32, tag="ps", bufs=8,
                                    name=f"pb{c}_{q}") for q in range(4)]
                    for j in range(KI):
                        lhs = actT[c][:, j, :]
                        for q in range(4):
                            nc.tensor.matmul(
                                pb[q][:], lhs,
                                wd4[j // 4][:, j % 4, q * 512:(q + 1) * 512],
                                start=(j == 0), stop=(j == KI - 1),
                            )
                    ot = work.tile([128, H], dt.bfloat16, tag="ot", bufs=2,
                                   name=f"ot{c}")
                    for q in range(4):
                        nc.vector.tensor_copy(ot[:, q * 512:(q + 1) * 512],
                                              pb[q][:])
                    # alternate output queues so the final drains overlap
                    eng = nc.gpsimd if c % 2 == 0 else nc.sync
                    eng.dma_start(outC[c * 128:(c + 1) * 128, :], ot[:])
                else:
                    # banks q0-q2 interleaved (stationary reused), then q3
                    # alone: q0-q2's copies+DMAs overlap q3's matmuls and the
                    # post-final-matmul tail is a single 512-wide copy+DMA.
                    pb = [psum.tile([128, 512], dt.float32, tag="ps", bufs=8,
                                    name=f"pb{c}_{q}") for q in range(4)]
                    for j in range(KI):
                        lhs = actT[c][:, j, :]
                        for q in range(3):
                            nc.tensor.matmul(
                                pb[q][:], lhs,
                                wd4[j // 4][:, j % 4, q * 512:(q + 1) * 512],
                                start=(j == 0), stop=(j == KI - 1),
                            )
                    for q in range(3):
                        otq = work.tile([128, 512], dt.bfloat16, tag="otl",
                                        bufs=4, name=f"ot{c}_{q}")
                        nc.scalar.activation(otq[:], pb[q][:], AF.Copy)
                        deng = nc.gpsimd if q % 2 == 0 else nc.sync
                        deng.dma_start(
                            outC[c * 128:(c + 1) * 128,
                                 q * 512:(q + 1) * 512], otq[:])
                    for j in range(KI):
                        nc.tensor.matmul(
                            pb[3][:], actT[c][:, j, :],
                            wd4[j // 4][:, j % 4, 3 * 512:4 * 512],
                            start=(j == 0), stop=(j == KI - 1),
                        )
                    otq = work.tile([128, 512], dt.bfloat16, tag="otl",
                                    bufs=4, name=f"ot{c}_3")
                    nc.vector.tensor_copy(otq[:], pb[3][:])
                    nc.sync.dma_start(
                        outC[c * 128:(c + 1) * 128, 3 * 512:4 * 512], otq[:])

    nc.compile()
    return nc


def _numpy_fallback(x, sel, scale, gate_up_weight, down_weight):
    """Correct host-side computation for overflow tokens (beyond the 512
    per-expert device capacity) and the pathological-skew full fallback."""
    wgu = np.asarray(gate_up_weight, dtype=np.float32)
    wdn = np.asarray(down_weight, dtype=np.float32)
    ii = wdn.shape[1]
    out = np.zeros_like(x)
    for e in range(wgu.shape[0]):
        tok = np.nonzero(sel == e)[0]
        if tok.size == 0:
            continue
        xsv = x[tok] * scale[tok][:, None]
        gu = xsv @ wgu[e]
        up, gate = gu[:, :ii], gu[:, ii:]
        out[tok] = (up * (gate / (1.0 + np.exp(-gate)))) @ wdn[e]
    return out


def kernel(hidden_states, gate_w, gate_up_weight, down_weight):
    from concourse.bass_utils import run_bass_kernel_spmd

    hs = np.asarray(hidden_states, dtype=np.float32)
    x = np.ascontiguousarray(hs).reshape(-1, H)
    nt = x.shape[0]
    gw = np.asarray(gate_w, dtype=np.float32)

    # Router (top-1): selected expert keeps sigmoid(logit), others contribute 0.
    logits = x @ gw.T                                   # [nt, E]
    sel = np.argmax(logits, axis=1)
    top = logits[np.arange(nt), sel]
    scale = (1.0 / (1.0 + np.exp(-top))).astype(np.float32)

    counts = np.bincount(sel, minlength=E)
    overflow = np.maximum(counts - C, 0)
    if int(overflow.sum()) > 1024:  # pathological skew; stay correct on host
        out = _numpy_fallback(x, sel, scale, gate_up_weight, down_weight)
        return out.reshape(hs.shape)
    counts_dev = np.minimum(counts, C)

    order = np.argsort(sel, kind="stable")
    offs = np.zeros(E + 1, dtype=np.int64)
    np.cumsum(counts, out=offs[1:])
    idx = np.zeros((E, C), dtype=np.int64)
    scale_pad = np.zeros((E, C), dtype=np.float32)
    over_tok = []
    for e in range(E):
        ce = int(counts_dev[e])
        idx[e, :ce] = order[offs[e]:offs[e] + ce]
        scale_pad[e, :ce] = scale[idx[e, :ce]]
        if int(counts[e]) > ce:
            over_tok.append(order[offs[e] + ce:offs[e] + int(counts[e])])

    gath = x[idx.reshape(-1)]                           # [E*C, H]
    gath *= scale_pad.reshape(-1, 1)
    gath_bf = gath.astype(ml_dtypes.bfloat16).reshape(E, C, H)
    # xsP[e, p, k*C + c] = x[c, k*128 + p]: partition-major (4 KB DMA rows)
    xsP_all = np.ascontiguousarray(
        gath_bf.reshape(E, C, H // 128, 128).transpose(0, 3, 2, 1)
    ).reshape(E, 128, (H // 128) * C)
    wgu_bf = np.asarray(gate_up_weight, dtype=np.float32).astype(ml_dtypes.bfloat16)
    wdn_bf = np.asarray(down_weight, dtype=np.float32).astype(ml_dtypes.bfloat16)

    if "prog" not in _PROGRAM_CACHE:
        _PROGRAM_CACHE["prog"] = _build_program()
    nc = _PROGRAM_CACHE["prog"]
    in_maps = [
        {"xsP": xsP_all[e], "wgu": wgu_bf[e], "wdn": wdn_bf[e]} for e in range(E)
    ]
    res = run_bass_kernel_spmd(nc, in_maps, list(range(E)))
    LAST["results"] = res
    LAST["C"] = C

    out = np.zeros((nt, H), dtype=np.float32)
    for e in range(E):
        ce = int(counts_dev[e])
        if ce:
            out[idx[e, :ce]] = np.asarray(
                res.results[e]["outC"][:ce, :], dtype=np.float32)
    if over_tok:
        ov = np.concatenate(over_tok)
        out[ov] = _numpy_fallback(
            x[ov], sel[ov], scale[ov], gate_up_weight, down_weight)
    return out.reshape(hs.shape)


# revision 24
# speedup vs baseline: 1.1268x; 1.1268x over previous
"""Top-1 MoE (BmmMoeModel) on 8 Trainium2 NeuronCores.

Strategy: expert-parallel with routing-aware dispatch.
  - Host computes the router (x @ gate_w.T, argmax, sigmoid) -- 0.26% of the
    model FLOPs -- and uses it as the sharding function: each core receives
    only the tokens routed to its expert (scaled by the routing weight,
    transposed to [H, C], cast to bf16) plus that expert's weights.
  - The final "sum over experts" is a disjoint scatter of each core's token
    outputs back into the full [T, H] output on the host (top-1 routing means
    non-selected experts contribute exactly zero).

On-device layout: both GEMMs run with the *activations as the stationary
operand* and the weights as the moving operand.  A stationary tile is reused
for 4 consecutive matmuls (the 4 512-wide PSUM banks covering the weight's
output columns), so the PE's between-matmul weight-reload bubble (~46 ns per
matmul when the stationary changes every matmul, measured) is amortized away
and the PE runs at its row-streaming rate.

  Phase A  gu[c,f] = sum_k x[c,k] wgu[k,f]: stationary = xsT[k-chunk,c-chunk]
           ([128,128]), moving = wgu[k-chunk,:] in 4 q-slices of 512.  PSUM
           out is [tokens, FF2] = 4 banks/c-chunk -> two passes of 2 c-chunks.
  silu-glu from PSUM (fp32): act[c, i] = up * silu(gate), stored bf16.
  transpose act [c,i] -> actT [i,c] with the DMA xbar (dma_start_transpose),
           off the critical engines.
  Phase B  out[c,h] = sum_i act[c,i] wdn[i,h]: stationary = actT[i-chunk,
           c-chunk], moving = wdn[i-chunk,:] in 4 q-slices.  Output lands
           token-major [C, H] fp32 -> contiguous 8 KB DMA rows and a
           transpose-free host scatter.

Weights stream k-tile-by-k-tile on the sync queue in the order Phase A
consumes them (accumulation over k is arrival-ordered); activations ride the
scalar-engine queue, outputs the gpsimd queue, so no DMA issue stream gates
another.  Matmuls run bf16 with fp32 PSUM accumulation (fp8 was measured at
5-6.5% relative error on this problem -- over the 2e-2 gate -- and rejected).
"""

import numpy as np
import ml_dtypes

B, S, H, I, E = 2, 2048, 2048, 1024, 8
T = B * S
FF2 = 2 * I
C = 512          # per-expert device token capacity (4 c-chunks of 128)

# Stash of the last run's BassKernelResults (for test harness introspection).
LAST = {}
_PROGRAM_CACHE = {}


def _build_program():
    import concourse.bass as bass
    import concourse.mybir as mybir
    import concourse.tile as tile
    from concourse import bacc

    dt = mybir.dt
    AF = mybir.ActivationFunctionType

    nc = bacc.Bacc(None, target_bir_lowering=False)
    # xsP[p, k*C + c] = x_tokens[c, k*128 + p]: partition-major so each DMA
    # row run is 4 KB (k-group of 4) instead of the 1 KB a plain [H, C]
    # layout would give -- 1 KB packets are per-packet-overhead-bound and
    # were measured to stall Phase A.
    xsP = nc.dram_tensor("xsP", [128, (H // 128) * C], dt.bfloat16,
                         kind="ExternalInput")
    wgu = nc.dram_tensor("wgu", [H, FF2], dt.bfloat16, kind="ExternalInput")
    wdn = nc.dram_tensor("wdn", [I, H], dt.bfloat16, kind="ExternalInput")
    outC = nc.dram_tensor("outC", [C, H], dt.float32, kind="ExternalOutput")

    KH = H // 128    # 16 contraction chunks for gate_up
    KI = I // 128    # 8 contraction chunks for down

    with tile.TileContext(nc) as tc:
        with (
            tc.tile_pool(name="res", bufs=1) as res,
            tc.tile_pool(name="work", bufs=1) as work,
            tc.tile_pool(name="psum", bufs=1, space=bass.MemorySpace.PSUM) as psum,
        ):
            # PE clock pre-warm: the HAM clock gate holds the PE at 1.2 GHz
            # until it has seen ~3.4 us of sustained activity; spin small
            # dependency-free matmuls so the ramp happens before real work.
            warm = work.tile([128, 128], dt.bfloat16, tag="warm", bufs=1,
                             name="warm")
            nc.gpsimd.memset(warm[:], 0.0)
            warm_ps = psum.tile([128, 64], dt.float32, tag="ps", bufs=8,
                                name="warm_ps")
            for _ in range(44):
                nc.tensor.matmul(warm_ps[0:32, :], warm[:, 0:32], warm[:, 0:64],
                                 start=True, stop=True)

            # All inputs ride ONE queue (sync), interleaved in exact
            # consumption order.  The DMA queues share the core's HBM
            # bandwidth, so a second concurrent input stream halves the
            # weight arrival rate and stalls Phase A (measured: +10 us);
            # strict FIFO on one queue keeps arrival == need order at full
            # bandwidth.  xs4[b] ([128,4,C], slice [:,j,:] = k-tile 4b+j)
            # is needed just before wg[4b].
            # The PE start is gated by the first stationary+moving tiles, so
            # the k=0 pieces are split small: xsA is one k-tile of tokens
            # (128 KB) and wg[0] arrives as two half-tiles.
            xsA = res.tile([128, 1, C], dt.bfloat16, tag="xsA", name="xsA")
            xsB = res.tile([128, 3, C], dt.bfloat16, tag="xsB", name="xsB")
            xs4 = [None] + [res.tile([128, 4, C], dt.bfloat16, tag=f"xs{b}",
                                     name=f"xs{b}") for b in range(1, 4)]
            wg0h = [res.tile([128, FF2 // 2], dt.bfloat16, tag=f"wg0{h}",
                             name=f"wg0{h}") for h in range(2)]
            wg = [None] + [res.tile([128, FF2], dt.bfloat16, tag=f"wg{k}",
                                    name=f"wg{k}") for k in range(1, KH)]
            xsP_r = xsP.rearrange("p (b j c) -> b p j c", b=4, j=4)

            def wg_dma(k):
                nc.sync.dma_start(wg[k][:], wgu[k * 128:(k + 1) * 128, :])

            nc.sync.dma_start(xsA[:], xsP_r[0][:, 0:1, :])
            nc.sync.dma_start(wg0h[0][:], wgu[0:128, 0:FF2 // 2])
            nc.sync.dma_start(wg0h[1][:], wgu[0:128, FF2 // 2:FF2])
            nc.sync.dma_start(xsB[:], xsP_r[0][:, 1:4, :])
            for k in range(1, 3):
                wg_dma(k)
            nc.sync.dma_start(xs4[1][:], xsP_r[1])
            for k in range(3, 8):
                wg_dma(k)
            nc.sync.dma_start(xs4[2][:], xsP_r[2])
            nc.sync.dma_start(xs4[3][:], xsP_r[3])
            for k in range(8, KH):
                wg_dma(k)

            def xs_sl(k, c):
                if k == 0:
                    return xsA[:, 0, c * 128:(c + 1) * 128]
                if k < 4:
                    return xsB[:, k - 1, c * 128:(c + 1) * 128]
                return xs4[k // 4][:, k % 4, c * 128:(c + 1) * 128]

            def wg_sl(k, q):
                if k == 0:
                    return wg0h[q // 2][:, (q % 2) * 512:(q % 2 + 1) * 512]
                return wg[k][:, q * 512:(q + 1) * 512]
            # down weight (Phase B moving): 2 tiles of 4 i-chunks each,
            # behind the Phase A stream.
            wd4 = []
            wdn_r = wdn.rearrange("(b j p) h -> b p j h", j=4, p=128)
            for b in range(2):
                t_ = res.tile([128, 4, H], dt.bfloat16, tag=f"wd{b}",
                              name=f"wd{b}")
                nc.sync.dma_start(t_[:], wdn_r[b])
                wd4.append(t_)

            act = [res.tile([128, I], dt.bfloat16, tag=f"act{c}",
                            name=f"act{c}") for c in range(4)]
            # actT[c][:, j, :] = act[c][:, j*128:(j+1)*128].T  (i-major)
            actT = [res.tile([128, KI, 128], dt.bfloat16, tag=f"actT{c}",
                             name=f"actT{c}") for c in range(4)]

            # Phase A, k-blocked with partial-sum spill: four blocks of
            # (c-pair x 8 k-steps).  Blocks over k=0..7 run while the second
            # half of the weight stream is still arriving and spill their
            # fp32 partials to SBUF; blocks over k=8..15 add the spill back
            # into PSUM (in place, on vector) before the silu-glu.  This
            # keeps the PE compute-bound through the weight-arrival window
            # instead of serializing a full-k pass behind the last weight
            # byte (~4 us).  Within each block the first c-chunk runs its
            # last SG k-steps alone so its spill/merge (the bank-freeing
            # consumers) overlap the second c-chunk's k-tail.
            sp = {(c, q): res.tile([128, 512], dt.float32, tag=f"sp{c}_{q}",
                                   name=f"sp{c}_{q}")
                  for c in range(4) for q in range(4)}
            KB = KH // 2
            SG = 3
            for half in range(2):
                ka, kb = half * KB, half * KB + KB
                for p in range(2):
                    cs = (2 * p, 2 * p + 1)
                    pa = {(c, q): psum.tile([128, 512], dt.float32, tag="ps",
                                            bufs=8, name=f"pa{half}{c}_{q}")
                          for c in cs for q in range(4)}

                    def a_mm(k, c):
                        lhs = xs_sl(k, c)
                        for q in range(4):
                            nc.tensor.matmul(
                                pa[(c, q)][:], lhs, wg_sl(k, q),
                                start=(k == ka), stop=(k == kb - 1),
                            )

                    def consume(c):
                        if half == 0:
                            # spill k0..7 partials, split scalar/vector
                            for q in range(4):
                                if q < 2:
                                    nc.scalar.activation(sp[(c, q)][:],
                                                         pa[(c, q)][:], AF.Copy)
                                else:
                                    nc.vector.tensor_copy(sp[(c, q)][:],
                                                          pa[(c, q)][:])
                        else:
                            # merge spill + silu-glu: up = q0,q1; gate = q2,q3
                            for h in range(2):
                                nc.vector.tensor_add(pa[(c, 2 + h)][:],
                                                     pa[(c, 2 + h)][:],
                                                     sp[(c, 2 + h)][:])
                                st = work.tile([128, 512], dt.float32,
                                               tag="silu", bufs=4,
                                               name=f"st{c}_{h}")
                                nc.scalar.activation(st[:], pa[(c, 2 + h)][:],
                                                     AF.Silu)
                                nc.vector.tensor_add(pa[(c, h)][:],
                                                     pa[(c, h)][:],
                                                     sp[(c, h)][:])
                                nc.vector.tensor_mul(
                                    act[c][:, h * 512:(h + 1) * 512],
                                    pa[(c, h)][:], st[:])
                            nc.sync.dma_start_transpose(actT[c][:], act[c][:])

                    for k in range(ka, kb - SG):
                        for c in cs:
                            a_mm(k, c)
                    for k in range(kb - SG, kb):
                        a_mm(k, cs[0])
                    consume(cs[0])
                    for k in range(kb - SG, kb):
                        a_mm(k, cs[1])
                    consume(cs[1])

            # Phase B: token-major out, one c-chunk at a time (4 banks).
            # The last c-chunk runs its output columns (q) outermost so the
            # trailing copy+DMA after the final matmul is only 512 wide.
            for c in range(4):
                if c < 3:
                    pb = [psum.tile([128, 512], dt.float32, tag="ps", bufs=8,
                                    name=f"pb{c}_{q}") for q in range(4)]
                    for j in range(KI):
                        lhs = actT[c][:, j, :]
                        for q in range(4):
                            nc.tensor.matmul(
                                pb[q][:], lhs,
                                wd4[j // 4][:, j % 4, q * 512:(q + 1) * 512],
                                start=(j == 0), stop=(j == KI - 1),
                            )
                    ot = work.tile([128, H], dt.float32, tag="ot", bufs=2,
                                   name=f"ot{c}")
                    for q in range(4):
                        nc.vector.tensor_copy(ot[:, q * 512:(q + 1) * 512],
                                              pb[q][:])
                    # alternate output queues so the final drains overlap
                    eng = nc.gpsimd if c % 2 == 0 else nc.sync
                    eng.dma_start(outC[c * 128:(c + 1) * 128, :], ot[:])
                else:
                    # banks q0-q2 interleaved (stationary reused), then q3
                    # alone: q0-q2's copies+DMAs overlap q3's matmuls and the
                    # post-final-matmul tail is a single 512-wide copy+DMA.
                    pb = [psum.tile([128, 512], dt.float32, tag="ps", bufs=8,
                                    name=f"pb{c}_{q}") for q in range(4)]
                    for j in range(KI):
                        lhs = actT[c][:, j, :]
                        for q in range(3):
                            nc.tensor.matmul(
                                pb[q][:], lhs,
                                wd4[j // 4][:, j % 4, q * 512:(q + 1) * 512],
                                start=(j == 0), stop=(j == KI - 1),
                            )
                    for q in range(3):
                        otq = work.tile([128, 512], dt.float32, tag="otl",
                                        bufs=4, name=f"ot{c}_{q}")
                        nc.vector.tensor_copy(otq[:], pb[q][:])
                        deng = nc.gpsimd if q % 2 == 0 else nc.sync
                        deng.dma_start(
                            outC[c * 128:(c + 1) * 128,
                                 q * 512:(q + 1) * 512], otq[:])
                    for j in range(KI):
                        nc.tensor.matmul(
                            pb[3][:], actT[c][:, j, :],
                            wd4[j // 4][:, j % 4, 3 * 512:4 * 512],
                            start=(j == 0), stop=(j == KI - 1),
                        )
                    otq = work.tile([128, 512], dt.float32, tag="otl",
                                    bufs=4, name=f"ot{c}_3")
                    nc.vector.tensor_copy(otq[:], pb[3][:])
                    nc.sync.dma_start(
                        outC[c * 128:(c + 1) * 128, 3 * 512:4 * 512], otq[:])

    nc.compile()
    return nc


def _numpy_fallback(x, sel, scale, gate_up_weight, down_weight):
    """Correct host-side computation for overflow tokens (beyond the 512
    per-expert device capacity) and the pathological-skew full fallback."""
    wgu = np.asarray(gate_up_weight, dtype=np.float32)
    wdn = np.asarray(down_weight, dtype=np.float32)
    ii = wdn.shape[1]
    out = np.zeros_like(x)
    for e in range(wgu.shape[0]):
        tok = np.nonzero(sel == e)[0]
        if tok.size == 0:
            continue
        xsv = x[tok] * scale[tok][:, None]
        gu = xsv @ wgu[e]
        up, gate = gu[:, :ii], gu[:, ii:]
        out[tok] = (up * (gate / (1.0 + np.exp(-gate)))) @ wdn[e]
    return out


def kernel(hidden_states, gate_w, gate_up_weight, down_weight):
    from concourse.bass_utils import run_bass_kernel_spmd

    hs = np.asarray(hidden_states, dtype=np.float32)
    x = np.ascontiguousarray(hs).reshape(-1, H)
    nt = x.shape[0]
    gw = np.asarray(gate_w, dtype=np.float32)

    # Router (top-1): selected expert keeps sigmoid(logit), others contribute 0.
    logits = x @ gw.T                                   # [nt, E]
    sel = np.argmax(logits, axis=1)
    top = logits[np.arange(nt), sel]
    scale = (1.0 / (1.0 + np.exp(-top))).astype(np.float32)

    counts = np.bincount(sel, minlength=E)
    overflow = np.maximum(counts - C, 0)
    if int(overflow.sum()) > 1024:  # pathological skew; stay correct on host
        out = _numpy_fallback(x, sel, scale, gate_up_weight, down_weight)
        return out.reshape(hs.shape)
    counts_dev = np.minimum(counts, C)

    order = np.argsort(sel, kind="stable")
    offs = np.zeros(E + 1, dtype=np.int64)
    np.cumsum(counts, out=offs[1:])
    idx = np.zeros((E, C), dtype=np.int64)
    scale_pad = np.zeros((E, C), dtype=np.float32)
    over_tok = []
    for e in range(E):
        ce = int(counts_dev[e])
        idx[e, :ce] = order[offs[e]:offs[e] + ce]
        scale_pad[e, :ce] = scale[idx[e, :ce]]
        if int(counts[e]) > ce:
            over_tok.append(order[offs[e] + ce:offs[e] + int(counts[e])])

    gath = x[idx.reshape(-1)]                           # [E*C, H]
    gath *= scale_pad.reshape(-1, 1)
    gath_bf = gath.astype(ml_dtypes.bfloat16).reshape(E, C, H)
    # xsP[e, p, k*C + c] = x[c, k*128 + p]: partition-major (4 KB DMA rows)
    xsP_all = np.ascontiguousarray(
        gath_bf.reshape(E, C, H // 128, 128).transpose(0, 3, 2, 1)
    ).reshape(E, 128, (H // 128) * C)
    wgu_bf = np.asarray(gate_up_weight, dtype=np.float32).astype(ml_dtypes.bfloat16)
    wdn_bf = np.asarray(down_weight, dtype=np.float32).astype(ml_dtypes.bfloat16)

    if "prog" not in _PROGRAM_CACHE:
        _PROGRAM_CACHE["prog"] = _build_program()
    nc = _PROGRAM_CACHE["prog"]
    in_maps = [
        {"xsP": xsP_all[e], "wgu": wgu_bf[e], "wdn": wdn_bf[e]} for e in range(E)
    ]
    res = run_bass_kernel_spmd(nc, in_maps, list(range(E)))
    LAST["results"] = res
    LAST["C"] = C

    out = np.zeros((nt, H), dtype=np.float32)
    for e in range(E):
        ce = int(counts_dev[e])
        if ce:
            out[idx[e, :ce]] = res.results[e]["outC"][:ce, :]
    if over_tok:
        ov = np.concatenate(over_tok)
        out[ov] = _numpy_fallback(
            x[ov], sel[ov], scale[ov], gate_up_weight, down_weight)
    return out.reshape(hs.shape)
